# revision 1
# baseline (speedup 1.0000x reference)
"""Multi-level-KV attention (MKA) kernel for 8 TRN2 NeuronCores.

Math shortcut: memory levels L2 (mean-pooled, all keys identical) and L3
(zeros) have exactly uniform attention, so their contributions collapse to
per-batch constant vectors folded into 3 augmented rows of the output
projection. Only L1 needs real attention.

Sharding: core c -> batch b=c//4, head-quad h=c%4 (4 of 16 heads; column
slice 256h:256h+256 of the q/k/v projections, row slice of Wo). The routing
MLP is column-sharded with a tiny [3,2048] logits AllReduce per batch group.
Each core returns a partial [T,C] output; the host sums each group of 4.

Softmax is computed without max-subtraction (scores are ~N(0,0.17) after
scale) and normalization is deferred: attention uses unnormalized exp with
the row-sum riding as a 65th ones-column in the v matmul; routing softmax's
denominator is applied once at the final PSUM drain.
"""
import sys
import types

import numpy as np

_REPO = "/opt/trn_rl_repo"

B, T, C, H = 2, 2048, 1024, 16
D = C // H
P = 128
SCALE = D ** -0.5
NHL = 4  # heads per core


def _setup_env():
    if _REPO not in sys.path:
        sys.path.insert(0, _REPO)
    import concourse.tile as tile
    from concourse import mybir
    from concourse.vector_clock import ScopedClock

    if getattr(tile.TileContext, "_drain_patched", False):
        return

    # This walrus build rejects CTRL instructions (Drain) carrying more than
    # one sync wait; move the end-of-kernel drain's waits onto 1-wait nops.
    def _drain_and_barrier_split(self, tick_clock, wait_clock):
        carrier = self.nc.sync.nop(nofuse=True, hint="drain_wait_carrier")
        wait_clock.add_sem_waits(
            carrier.ins, ScopedClock({None: tick_clock.global_clock})
        )
        si = carrier.ins.sync_info
        waits = list(si.on_wait) if si is not None and si.on_wait else []
        if si is not None:
            si.on_wait = waits[:1]
        for w in waits[1:]:
            nop = self.nc.sync.nop(nofuse=True, hint="drain_wait_carrier")
            nop.ins.sync_info = mybir.SyncInfo(on_wait=[w], on_update=[])
        self.nc.sync.drain()
        self.nc.all_engine_barrier()
        assert self.sems is not None
        popped = self.nc._tile_sem_poison_stack.pop()
        assert popped is self._sem_poison
        self.nc.clear_and_free_semaphores(list(self.sems.allocated().values()))
        self.nc.all_engine_barrier()

    tile.TileContext._drain_and_barrier = _drain_and_barrier_split
    tile.TileContext._drain_patched = True


def build_nc():
    _setup_env()
    from contextlib import ExitStack

    import concourse.bass as bass
    import concourse.tile as tile
    from concourse import mybir

    f32 = mybir.dt.float32
    bf = mybir.dt.bfloat16
    Exp = mybir.ActivationFunctionType.Exp
    Tanh = mybir.ActivationFunctionType.Tanh
    Copy = mybir.ActivationFunctionType.Copy

    nc = bass.Bass()
    xT = nc.dram_tensor("xT", [C, T], bf, kind="ExternalInput")
    wq = nc.dram_tensor("wq", [C, 256], bf, kind="ExternalInput")
    wk = nc.dram_tensor("wk", [C, 256], bf, kind="ExternalInput")
    wv = nc.dram_tensor("wv", [C, 256], bf, kind="ExternalInput")
    wr1 = nc.dram_tensor("wr1", [C, 256], bf, kind="ExternalInput")
    wr2 = nc.dram_tensor("wr2", [256, 3], bf, kind="ExternalInput")
    wo = nc.dram_tensor("wo", [256, C], bf, kind="ExternalInput")
    crows = nc.dram_tensor("crows", [3, C], bf, kind="ExternalInput")
    bq2 = nc.dram_tensor("bq2", [2, P, 1], f32, kind="ExternalInput")
    br1t = nc.dram_tensor("br1t", [2, P, 1], f32, kind="ExternalInput")
    br2z = nc.dram_tensor("br2z", [3, 1], f32, kind="ExternalInput")
    outd = nc.dram_tensor("out", [T, C], f32, kind="ExternalOutput")

    with tile.TileContext(nc) as tc, ExitStack() as ctx:
        pp = ctx.enter_context(tc.tile_pool(name="persist", bufs=1))
        exp_pool = ctx.enter_context(tc.tile_pool(name="expool", bufs=6))
        scp = ctx.enter_context(tc.tile_pool(name="scp", bufs=1, space="PSUM"))
        upp = ctx.enter_context(tc.tile_pool(name="upp", bufs=4, space="PSUM"))
        stg = ctx.enter_context(tc.tile_pool(name="stg", bufs=3))
        drp = ctx.enter_context(tc.tile_pool(name="drm", bufs=1, space="DRAM"))

        # ---- persistent loads --------------------------------------------
        xt = []
        for i in range(8):
            t_ = pp.tile([P, T], bf, name=f"xt{i}")
            nc.sync.dma_start(out=t_, in_=xT[i * P:(i + 1) * P, :])
            xt.append(t_)

        def wload(w_, nm):
            ts_ = []
            for i in range(8):
                t_ = pp.tile([P, 256], bf, name=f"{nm}{i}")
                nc.sync.dma_start(out=t_, in_=w_[i * P:(i + 1) * P, :])
                ts_.append(t_)
            return ts_

        wq_s = wload(wq, "wqs")
        wk_s = wload(wk, "wks")
        wv_s = wload(wv, "wvs")
        wr1_s = wload(wr1, "wr1s")
        wo_s = []
        for j in range(2):
            t_ = pp.tile([P, C], bf, name=f"wos{j}")
            nc.sync.dma_start(out=t_, in_=wo[j * P:(j + 1) * P, :])
            wo_s.append(t_)
        crow_sb = pp.tile([3, C], bf, name="crow_sb")
        nc.sync.dma_start(out=crow_sb, in_=crows[:, :])
        wr2_sb = pp.tile([P, 2, 3], bf, name="wr2_sb")
        nc.sync.dma_start(
            out=wr2_sb, in_=wr2.rearrange("(a p) b -> p a b", p=P)
        )
        bq_sb = pp.tile([P, 2], f32, name="bq_sb")
        nc.sync.dma_start(out=bq_sb, in_=bq2.rearrange("a p o -> p (a o)"))
        br1_sb = pp.tile([P, 2], f32, name="br1_sb")
        nc.sync.dma_start(out=br1_sb, in_=br1t.rearrange("a p o -> p (a o)"))
        br2_sb = pp.tile([3, 1], f32, name="br2_sb")
        nc.sync.dma_start(out=br2_sb, in_=br2z[:, :])

        qT2 = [pp.tile([P, T], bf, name=f"qT{p}") for p in range(2)]
        kT2 = [pp.tile([P, T], bf, name=f"kT{p}") for p in range(2)]
        attT = [pp.tile([P, T], bf, name=f"attT{p}") for p in range(2)]
        vt = [pp.tile([P, NHL, 65], bf, name=f"vt{j}") for j in range(16)]
        gt = [pp.tile([P, T], bf, name=f"gt{m}") for m in range(2)]
        e_f = pp.tile([3, T], f32, name="e_f")
        e_b = pp.tile([3, T], bf, name="e_b")
        lg_sb = pp.tile([3, T], f32, name="lg_sb")
        lgr_sb = pp.tile([3, T], f32, name="lgr_sb")
        s3v = pp.tile([1, T], f32, name="s3v")
        r1v = pp.tile([1, T], f32, name="r1v")
        r2v = pp.tile([1, T], f32, name="r2v")
        s3rt = pp.tile([P, 16], f32, name="s3rt")
        ones1 = pp.tile([1, 64], bf, name="ones1")
        nc.vector.memset(ones1, 1.0)

        # ---- routing: gT = tanh(Wr1_blk^T xT), partial logits, AllReduce --
        for m in range(2):
            for ch in range(4):
                g_ps = upp.tile([P, 512], f32, tag="u", name="g_ps")
                for kt in range(8):
                    nc.tensor.matmul(
                        g_ps,
                        wr1_s[kt][:, m * P:(m + 1) * P],
                        xt[kt][:, ch * 512:(ch + 1) * 512],
                        start=(kt == 0),
                        stop=(kt == 7),
                    )
                nc.scalar.activation(
                    out=gt[m][:, ch * 512:(ch + 1) * 512],
                    in_=g_ps,
                    func=Tanh,
                    bias=br1_sb[:, m:m + 1],
                    scale=1.0,
                )
        for ch in range(4):
            lg_ps = upp.tile([3, 512], f32, tag="u", name="lg_ps")
            for a in range(2):
                nc.tensor.matmul(
                    lg_ps,
                    wr2_sb[:, a, :],
                    gt[a][:, ch * 512:(ch + 1) * 512],
                    start=(a == 0),
                    stop=(a == 1),
                )
            nc.vector.tensor_scalar_add(
                out=lg_sb[:, ch * 512:(ch + 1) * 512],
                in0=lg_ps,
                scalar1=br2_sb,
            )
        lg_in = drp.tile([3, T], f32, name="lg_in")
        lg_out = drp.tile([3, T], f32, name="lg_out")
        nc.sync.dma_start(out=lg_in, in_=lg_sb)
        nc.gpsimd.collective_compute(
            "AllReduce",
            mybir.AluOpType.add,
            replica_groups=[[0, 1, 2, 3], [4, 5, 6, 7]],
            ins=[lg_in.opt()],
            outs=[lg_out.opt()],
        )
        nc.sync.dma_start(out=lgr_sb, in_=lg_out)
        nc.scalar.activation(out=e_f, in_=lgr_sb, func=Exp, scale=1.0)
        nc.vector.tensor_copy(out=e_b, in_=e_f)
        nc.sync.dma_start(out=r1v, in_=e_f[1:2, :])
        nc.sync.dma_start(out=r2v, in_=e_f[2:3, :])
        nc.vector.tensor_add(out=s3v, in0=e_f[0:1, :], in1=r1v)
        nc.vector.tensor_add(out=s3v, in0=s3v, in1=r2v)
        nc.vector.reciprocal(out=s3v, in_=s3v)
        s3d = drp.tile([1, T], f32, name="s3d")
        nc.sync.dma_start(out=s3d, in_=s3v)
        nc.sync.dma_start(
            out=s3rt, in_=s3d.rearrange("a (i p) -> p (a i)", p=P)
        )

        # ---- projections: qT/kT (heads stacked in pairs), v_aug ----------
        for p_ in range(2):
            for ch in range(4):
                q_ps = upp.tile([P, 512], f32, tag="u", name="q_ps")
                for kt in range(8):
                    nc.tensor.matmul(
                        q_ps,
                        wq_s[kt][:, p_ * P:(p_ + 1) * P],
                        xt[kt][:, ch * 512:(ch + 1) * 512],
                        start=(kt == 0),
                        stop=(kt == 7),
                    )
                nc.vector.tensor_scalar_add(
                    out=qT2[p_][:, ch * 512:(ch + 1) * 512],
                    in0=q_ps,
                    scalar1=bq_sb[:, p_:p_ + 1],
                )
                k_ps = upp.tile([P, 512], f32, tag="u", name="k_ps")
                for kt in range(8):
                    nc.tensor.matmul(
                        k_ps,
                        wk_s[kt][:, p_ * P:(p_ + 1) * P],
                        xt[kt][:, ch * 512:(ch + 1) * 512],
                        start=(kt == 0),
                        stop=(kt == 7),
                    )
                nc.vector.tensor_copy(
                    out=kT2[p_][:, ch * 512:(ch + 1) * 512], in_=k_ps
                )
        for j in range(16):
            v_ps = upp.tile([P, 256], f32, tag="u", name="v_ps")
            for kt in range(8):
                nc.tensor.matmul(
                    v_ps,
                    xt[kt][:, j * P:(j + 1) * P],
                    wv_s[kt],
                    start=(kt == 0),
                    stop=(kt == 7),
                )
            nc.vector.tensor_copy(
                out=vt[j][:, :, 0:64],
                in_=v_ps.rearrange("p (h d) -> p h d", h=NHL),
            )
            nc.vector.memset(vt[j][:, :, 64:65], 1.0)

        # ---- attention ----------------------------------------------------
        for p_ in range(2):
            for half in range(2):
                u_ps = [
                    upp.tile([65, 512], f32, tag="u", name="u_ps")
                    for _ in range(4)
                ]
                for kt in range(16):
                    sc0 = scp.tile([P, 1024], f32, tag="sc0", name="sc0")
                    sc1 = scp.tile([P, 1024], f32, tag="sc1", name="sc1")
                    for n2 in range(2):
                        c0 = half * 1024 + n2 * 512
                        nc.tensor.matmul(
                            sc0[:, n2 * 512:(n2 + 1) * 512],
                            kT2[p_][0:64, kt * P:(kt + 1) * P],
                            qT2[p_][0:64, c0:c0 + 512],
                            start=True,
                            stop=True,
                        )
                        nc.tensor.matmul(
                            sc1[:, n2 * 512:(n2 + 1) * 512],
                            kT2[p_][64:128, kt * P:(kt + 1) * P],
                            qT2[p_][64:128, c0:c0 + 512],
                            start=True,
                            stop=True,
                        )
                    ex0 = exp_pool.tile([P, 1024], bf, tag="ex", name="ext")
                    nc.scalar.activation(out=ex0, in_=sc0, func=Exp, scale=SCALE)
                    ex1 = exp_pool.tile([P, 1024], bf, tag="ex", name="ext")
                    nc.scalar.activation(out=ex1, in_=sc1, func=Exp, scale=SCALE)
                    exs = (ex0, ex1)
                    for h2 in range(2):
                        for qc in range(2):
                            nc.tensor.matmul(
                                u_ps[h2 * 2 + qc],
                                vt[kt][:, p_ * 2 + h2, :],
                                exs[h2][:, qc * 512:(qc + 1) * 512],
                                start=(kt == 0),
                                stop=(kt == 15),
                            )
                for h2 in range(2):
                    for qc in range(2):
                        up_ = u_ps[h2 * 2 + qc]
                        c0 = half * 1024 + qc * 512
                        usb = stg.tile([65, 512], f32, name="usb")
                        nc.vector.tensor_copy(out=usb, in_=up_)
                        su = stg.tile([1, 512], f32, name="su")
                        nc.sync.dma_start(out=su, in_=usb[64:65, :])
                        nc.vector.reciprocal(out=su, in_=su)
                        w1 = stg.tile([1, 512], bf, name="w1")
                        nc.vector.tensor_mul(
                            out=w1, in0=su, in1=e_f[0:1, c0:c0 + 512]
                        )
                        wb = upp.tile([64, 512], f32, tag="u", name="wb")
                        nc.tensor.matmul(
                            wb, ones1, w1, start=True, stop=True
                        )
                        mo = stg.tile([64, 512], bf, name="mo")
                        nc.vector.tensor_mul(
                            out=mo, in0=usb[0:64, :], in1=wb
                        )
                        nc.sync.dma_start(
                            out=attT[p_][h2 * 64:(h2 + 1) * 64, c0:c0 + 512],
                            in_=mo,
                        )

        # ---- output projection (augmented with routing const rows) -------
        for qt in range(16):
            po = scp.tile(
                [P, 1024], f32, tag=("sc0" if qt % 2 == 0 else "sc1"), name="po"
            )
            for n2 in range(2):
                ns = slice(n2 * 512, (n2 + 1) * 512)
                nc.tensor.matmul(
                    po[:, ns],
                    attT[0][:, qt * P:(qt + 1) * P],
                    wo_s[0][:, ns],
                    start=True,
                    stop=False,
                )
                nc.tensor.matmul(
                    po[:, ns],
                    attT[1][:, qt * P:(qt + 1) * P],
                    wo_s[1][:, ns],
                    start=False,
                    stop=False,
                )
                nc.tensor.matmul(
                    po[:, ns],
                    e_b[:, qt * P:(qt + 1) * P],
                    crow_sb[:, ns],
                    start=False,
                    stop=True,
                )
            ou = stg.tile([P, C], f32, name="ou")
            nc.vector.tensor_scalar_mul(
                out=ou, in0=po, scalar1=s3rt[:, qt:qt + 1]
            )
            nc.sync.dma_start(out=outd[qt * P:(qt + 1) * P, :], in_=ou)

    _split_excess_waits(nc, mybir)
    return nc


def _split_excess_waits(nc, mybir, keep=1):
    """Walrus in this build accepts at most 1 sync wait per instruction;
    move excess waits onto single-wait nops inserted just before, on the
    same engine (same-engine program order makes this equivalent)."""
    n_extra = 0
    for bb in nc.main_func.blocks:
        out = []
        for inst in bb.instructions:
            si = inst.sync_info
            if si is not None and si.on_wait and len(si.on_wait) > keep:
                waits = list(si.on_wait)
                si.on_wait = waits[:keep]
                for w in waits[keep:]:
                    n_extra += 1
                    nop = mybir.InstNoOp(
                        name=f"wsplit_{n_extra}_{inst.name}",
                        engine=inst.engine,
                        ins=[],
                        outs=[],
                        sync_info=mybir.SyncInfo(on_wait=[w], on_update=[]),
                    )
                    out.append(nop)
            out.append(inst)
        bb.instructions[:] = out


def make_in_maps(inputs):
    import ml_dtypes

    bf16 = ml_dtypes.bfloat16
    f32 = np.float32
    x = np.asarray(inputs["hidden_states"], f32)
    Wq = np.asarray(inputs["Wq"], f32)
    bq = np.asarray(inputs["bq"], f32)
    Wk = np.asarray(inputs["Wk"], f32)
    Wv = np.asarray(inputs["Wv"], f32)
    bv = np.asarray(inputs["bv"], f32)
    Wo = np.asarray(inputs["Wo"], f32)
    bo = np.asarray(inputs["bo"], f32)
    Wr1 = np.asarray(inputs["Wr1"], f32)
    br1 = np.asarray(inputs["br1"], f32)
    Wr2 = np.asarray(inputs["Wr2"], f32)
    br2 = np.asarray(inputs["br2"], f32)

    xTb = [np.ascontiguousarray(x[b].T).astype(bf16) for b in range(B)]
    c3 = bv @ Wo + bo
    c2 = []
    for b in range(B):
        mean = x[b].mean(axis=0)
        c2.append((mean @ Wv + bv) @ Wo + bo)

    def cut(w, cs, ce):
        return np.ascontiguousarray(w[:, cs:ce]).astype(bf16)

    in_maps = []
    for c in range(8):
        b, h = divmod(c, 4)
        cs, ce = h * 256, h * 256 + 256
        crows_np = np.zeros((3, C), f32)
        crows_np[0] = bv[cs:ce] @ Wo[cs:ce, :] + (bo if h == 0 else 0.0)
        if h == 0:
            crows_np[1] = c2[b]
            crows_np[2] = c3
        in_maps.append({
            "xT": xTb[b],
            "wq": cut(Wq, cs, ce),
            "wk": cut(Wk, cs, ce),
            "wv": cut(Wv, cs, ce),
            "wr1": cut(Wr1, cs, ce),
            "wr2": np.ascontiguousarray(Wr2[cs:ce, :]).astype(bf16),
            "wo": np.ascontiguousarray(Wo[cs:ce, :]).astype(bf16),
            "crows": crows_np.astype(bf16),
            "bq2": np.ascontiguousarray(bq[cs:ce].reshape(2, P, 1)),
            "br1t": np.ascontiguousarray(br1[cs:ce].reshape(2, P, 1)),
            "br2z": np.ascontiguousarray(
                (br2 if h == 0 else np.zeros(3, f32)).reshape(3, 1)
            ),
        })
    return in_maps


_NC = None


def kernel(**inputs):
    global _NC
    _setup_env()
    from concourse.bass_utils import run_bass_kernel_spmd

    if _NC is None:
        _NC = build_nc()
    in_maps = make_in_maps(inputs)
    res = run_bass_kernel_spmd(_NC, in_maps, core_ids=list(range(8)))
    outs = [res.results[c]["out"] for c in range(8)]
    full = np.stack(
        [outs[0] + outs[1] + outs[2] + outs[3],
         outs[4] + outs[5] + outs[6] + outs[7]],
        axis=0,
    )
    return full.astype(np.float32)

